# revision 13
# baseline (speedup 1.0000x reference)
"""Bass/TRN2 kernel for nn_BiRNNLayers: 2-layer BiLSTM (B=64, T=512, H=128,
vocab 50000) with masked Keras-style scan, feature pooling and FC head.

Strategy (8 NeuronCores, data-parallel over batch, 8 rows/core):
- Embedding gather on device (indirect DMA), PE transpose to H-on-partitions.
- Merged-direction scan: one [128, 4(gate), 2(dir), BC] tile per step; the
  xp term is injected into PSUM by an identity matmul (start=True) and the 8
  per-gate recurrent matmuls accumulate on top (start=False) -> the vector
  add drops off the critical chain.
- All activations share one (Tanh, scale=0.5) signature so the scalar engine
  never reloads its activation table; sigmoid gates are tanh(z/2) with the
  column scaling folded into weights; state kept as H'=2h, C'=2c.
- bf16 weights/hidden state (4x cheaper LDWEIGHTS + matmul); cell math fp32.
- Masked carry: C-carry exact via gate saturation (+-KSAT folded into xp),
  H-carry via copy_predicated with a u8 mask.
- Everything lives in SBUF (xp, y0, y1); no per-step DMA. The backward
  direction is stored scan-order; time reversal is done with reversed SBUF
  views (matmul rhs) and a 128x128 permutation matmul in the pooling stage.
"""
import numpy as np
import ml_dtypes

import concourse.bass as bass
import concourse.mybir as mybir
import concourse.tile as tile
import bass_rust

P = 128
T = 512
H = 128
E = 128
B_FULL = 64
NCORES = 8
BC = B_FULL // NCORES  # batch rows per core
VOCAB = 50000
NCLS = 10
KSAT = 40.0            # xp saturation offset for masked steps (tanh arg +-20)
UNROLL = 8

AF = mybir.ActivationFunctionType
ALU = mybir.AluOpType
dt = mybir.dt
bf16 = ml_dtypes.bfloat16

_hook_installed = False


def _install_hook():
    """Surface compile-hook tracebacks (PJRT swallows them otherwise)."""
    global _hook_installed
    if _hook_installed:
        return
    _hook_installed = True
    import traceback
    import concourse.bass2jax as bass2jax
    import libneuronxla

    orig = bass2jax.neuronx_cc_hook

    def dbg_hook(*a, **k):
        try:
            return orig(*a, **k)
        except BaseException:
            traceback.print_exc()
            raise

    bass2jax.neuronx_cc_hook = dbg_hook
    if not hasattr(libneuronxla, "orig_neuronx_cc"):
        libneuronxla.orig_neuronx_cc = libneuronxla.neuronx_cc
    libneuronxla.neuronx_cc = dbg_hook


def split_multi_waits(nc):
    """This container's walrus encodes at most one sem wait per instruction;
    hoist extra waits onto preceding same-engine NoOps."""
    for fn in nc.m.functions:
        for bb in fn.blocks:
            out = []
            changed = False
            for inst in bb.instructions:
                si = inst.sync_info
                waits = list(si.on_wait) if si is not None and si.on_wait else []
                if len(waits) > 1:
                    changed = True
                    for k, w in enumerate(waits[:-1]):
                        nop = mybir.InstNoOp(name=f"{inst.name}-sw{k}")
                        nop.engine = inst.engine
                        nop.sync_info = bass_rust.SyncInfo(on_wait=[w], on_update=[])
                        out.append(nop)
                    inst.sync_info = bass_rust.SyncInfo(
                        on_wait=[waits[-1]], on_update=list(si.on_update)
                    )
                out.append(inst)
            if changed:
                bb.instructions = out


# ---------------------------------------------------------------------------
# host-side weight folding
# ---------------------------------------------------------------------------

def _fold_weights(inputs):
    f32 = np.float32
    # all activations are tanh(0.5*stored): sigmoid gates (i,f,o) need
    # stored = z (cs=1), the g gate needs stored = 2*z (cs=2)
    cs2 = np.concatenate([
        np.ones(H), np.ones(H), np.full(H, 2.0), np.ones(H)
    ]).astype(f32)

    w = {}
    for l in (0, 1):
        for d in ("f", "b"):
            Wx = np.asarray(inputs[f"Wx_{d}{l}"], f32)
            Wh = np.asarray(inputs[f"Wh_{d}{l}"], f32)
            b = np.asarray(inputs[f"b_{d}{l}"], f32)
            w[f"wh{l}{d}"] = ((Wh * 0.5) * cs2).astype(bf16)
            be = (b * cs2).astype(f32)
            w[f"bcol{l}{d}"] = np.ascontiguousarray(
                be.reshape(4, H).T)  # [128, 4]
            if l == 0:
                w[f"wx0{d}"] = (Wx * cs2).astype(bf16)
            else:
                # rows 0:128 multiply y0f = 2*hf, rows 128:256 multiply y0b
                w[f"wx1{d}f"] = ((Wx[0:H] * 0.5) * cs2).astype(bf16)
                w[f"wx1{d}b"] = ((Wx[H:2 * H] * 0.5) * cs2).astype(bf16)

    w["emb"] = np.asarray(inputs["emb"], f32)

    fcw = np.asarray(inputs["fc_W"], f32).copy()  # [2T, 10]
    fcw[:T] *= 0.5          # mx rows: feat carries 2*mx
    fcw[T:] *= 1.0 / 512.0  # av rows: feat carries sum(2h) over 256 feats
    w["fcw"] = fcw.astype(f32)
    w["fcb_rep"] = np.tile(np.asarray(inputs["fc_b"], f32)[None, :], (BC, 1))
    w["ident"] = np.eye(P, dtype=f32)
    w["idb"] = np.eye(P, dtype=f32).astype(bf16)
    # pooling-time partition permutation for the scan-order backward dir:
    # token partition p=(t_local*8+b) maps to p'=(15-t_local)*8+b
    pm = np.zeros((P, P), dtype=f32)
    for k in range(P):
        pm[k, (15 - k // BC) * BC + k % BC] = 1.0
    w["perm"] = pm
    return w


# ---------------------------------------------------------------------------
# device program
# ---------------------------------------------------------------------------

def _build():
    nc = bass.Bass("TRN2", target_bir_lowering=False, debug=False,
                   num_devices=NCORES)

    def di(name, shape, dtype=dt.float32):
        return nc.dram_tensor(name, shape, dtype, kind="ExternalInput")

    emb_d = di("emb", [VOCAB + 1, E])
    ident_d = di("ident", [P, P])
    idb_d = di("idb", [P, P], dt.bfloat16)
    perm_d = di("perm", [P, P])
    idx_d = di("idx", [T * BC], dt.int32)
    m2_d = di("m2", [P, T, 2, BC], dt.uint8)
    fcw_d = di("fcw", [2 * T, NCLS])
    fcb_d = di("fcb_rep", [BC, NCLS])
    wdram = {}
    for l in (0, 1):
        for d in ("f", "b"):
            wdram[f"wh{l}{d}"] = di(f"wh{l}{d}", [H, 4 * H], dt.bfloat16)
            wdram[f"bcol{l}{d}"] = di(f"bcol{l}{d}", [P, 4])
            if l == 0:
                wdram[f"wx0{d}"] = di(f"wx0{d}", [E, 4 * H], dt.bfloat16)
            else:
                wdram[f"wx1{d}f"] = di(f"wx1{d}f", [H, 4 * H], dt.bfloat16)
                wdram[f"wx1{d}b"] = di(f"wx1{d}b", [H, 4 * H], dt.bfloat16)

    out_d = nc.dram_tensor("out", [BC, NCLS], dt.float32, kind="ExternalOutput")
    feat_dram = nc.dram_tensor("feat", [2, T, BC], dt.float32)

    NTOK = T * BC            # 4096 tokens per core
    NCH = NTOK // P          # 32 gather/pool chunks
    NXC = NTOK // 512        # 8 xp matmul chunks
    TCH = 512 // BC          # 64 timesteps per xp chunk
    TPC = P // BC            # 16 timesteps per pooling chunk

    with tile.TileContext(nc) as tc:
        with (
            tc.tile_pool(name="const", bufs=1) as cpool,
            tc.tile_pool(name="work", bufs=4) as wpool,
            tc.tile_pool(name="psx", bufs=2, space="PSUM") as psx,
            tc.tile_pool(name="psz", bufs=4, space="PSUM") as psz,
            tc.tile_pool(name="psf", bufs=1, space="PSUM") as psf,
        ):
            # ---- constant loads
            ident = cpool.tile([P, P], dt.float32, tag="ident")
            nc.sync.dma_start(out=ident[:], in_=ident_d[:])
            idb = cpool.tile([P, P], dt.bfloat16, tag="idb")
            nc.sync.dma_start(out=idb[:], in_=idb_d[:])
            perm = cpool.tile([P, P], dt.float32, tag="perm")
            nc.sync.dma_start(out=perm[:], in_=perm_d[:])
            idx_t = cpool.tile([P, NCH], dt.int32, tag="idx")
            nc.sync.dma_start(
                out=idx_t[:], in_=idx_d.rearrange("(c p) -> p c", p=P))
            m2 = cpool.tile([P, T, 2, BC], dt.uint8, tag="m2", name="m2")
            nc.sync.dma_start(out=m2[:], in_=m2_d[:])
            wsb = {}
            for k, dr in wdram.items():
                sh = list(dr.shape)
                wt_ = cpool.tile(sh, dr.dtype, tag=k, name=k)
                nc.sync.dma_start(out=wt_[:], in_=dr[:])
                wsb[k] = wt_
            fcw_t = cpool.tile([P, 2 * T // P, NCLS], dt.float32, tag="fcw")
            nc.sync.dma_start(
                out=fcw_t[:], in_=fcw_d.rearrange("(q p) c -> p q c", p=P))
            fcb_t = cpool.tile([BC, NCLS], dt.float32, tag="fcb")
            nc.sync.dma_start(out=fcb_t[:], in_=fcb_d[:])

            # persistent SBUF state
            xpT = cpool.tile([P, T, 4, 2, BC], dt.bfloat16, tag="xpT",
                             name="xpT")
            y0 = cpool.tile([P, 2, T, BC], dt.bfloat16, tag="y0", name="y0")
            y1 = cpool.tile([P, 2, T, BC], dt.float32, tag="y1", name="y1")
            # rolling H history for one UNROLL block; slot j = h after step j
            hwin = cpool.tile([P, UNROLL, 2, BC], dt.bfloat16, tag="hwin",
                              name="hwin")
            Cs = cpool.tile([P, 2, BC], dt.float32, tag="Cs", name="Cs")

            def xp_epilogue(l, dd, d, n, g, ps):
                """xpT[:, chunk, g, dd, :] = ps + bias_col + K_g*(1-m)."""
                t0, t1 = n * TCH, (n + 1) * TCH
                dst = xpT[:, t0:t1, g, dd, :]
                bcol = wsb[f"bcol{l}{d}"]
                kg = -KSAT if g == 0 else (KSAT if g == 1 else 0.0)
                if kg != 0.0:
                    # tmp = m*(-kg) + ps in fp32; the big +-KSAT intermediate
                    # must not round through bf16 (ulp(40) = 0.25), so only
                    # the final small-valued sum is written to the bf16 dst
                    tmp = wpool.tile([P, TCH, BC], dt.float32, tag="xptmp")
                    nc.vector.scalar_tensor_tensor(
                        out=tmp[:], in0=m2[:, t0:t1, dd, :], scalar=-kg,
                        in1=ps[:], op0=ALU.mult, op1=ALU.add)
                    nc.vector.tensor_scalar(
                        out=dst, in0=tmp[:], scalar1=bcol[:, g:g + 1],
                        scalar2=float(kg), op0=ALU.add, op1=ALU.add)
                else:
                    nc.vector.tensor_scalar(
                        out=dst, in0=ps[:], scalar1=bcol[:, g:g + 1],
                        scalar2=None, op0=ALU.add)

            # ---- embedding gather + transpose + layer-0 xp
            with tc.tile_pool(name="gph", bufs=3) as gpool, \
                 tc.tile_pool(name="gbig", bufs=1) as gbig:
                g128 = gbig.tile([P, T, BC], dt.bfloat16, tag="g128")
                g128f = g128[:].rearrange("p t b -> p (t b)")
                for c in range(NCH):
                    gr = gpool.tile([P, E], dt.float32, tag="gr")
                    nc.gpsimd.indirect_dma_start(
                        out=gr[:], out_offset=None, in_=emb_d[:],
                        in_offset=bass.IndirectOffsetOnAxis(
                            ap=idx_t[:, c:c + 1], axis=0),
                    )
                    pt = psx.tile([P, P], dt.float32, tag="psxp")
                    nc.tensor.transpose(out=pt[:], in_=gr[:], identity=ident[:])
                    nc.vector.tensor_copy(
                        out=g128f[:, c * P:(c + 1) * P], in_=pt[:])

                for dd, d, rv in ((0, "f", g128[:]), (1, "b", g128[:, ::-1, :])):
                    wxa = wsb[f"wx0{d}"]
                    for n in range(NXC):
                        t0, t1 = n * TCH, (n + 1) * TCH
                        for g in range(4):
                            ps = psx.tile([P, TCH, BC], dt.float32, tag="psxp")
                            nc.tensor.matmul(
                                out=ps[:], lhsT=wxa[:, g * H:(g + 1) * H],
                                rhs=rv[:, t0:t1, :], start=True, stop=True)
                            xp_epilogue(0, dd, d, n, g, ps)

            # ---- the merged-direction scan
            def scan_layer(l, ysb):
                nc.vector.memset(hwin[:], 0.0)
                nc.vector.memset(Cs[:], 0.0)
                whf = wsb[f"wh{l}f"]
                whb = wsb[f"wh{l}b"]
                with tc.For_i(0, T, UNROLL, staggered_reset=True) as t0:
                    # one dynamic AP per engine per block (per-step ds()
                    # offsets exhaust the register file); everything inside
                    # the block indexes statically
                    xw = xpT[:, bass.ds(t0, UNROLL), :, :, :]
                    mw = m2[:, bass.ds(t0, UNROLL), :, :]
                    yw = ysb[:, :, bass.ds(t0, UNROLL), :]
                    # stage the block's xp in SBUF (only the DVE handles
                    # dynamic APs); each step injects it into PSUM via an
                    # identity matmul so the accumulation group is pure-PE
                    # (mixed DVE-write + PE-accumulate on a PSUM bank races)
                    xstage = wpool.tile([P, UNROLL, 4, 2, BC], dt.bfloat16,
                                        tag="xstage")
                    nc.vector.tensor_copy(out=xstage[:], in_=xw)
                    zsblk = psz.tile([P, UNROLL, 4, 2, BC], dt.float32,
                                     tag="zs")
                    mwin = wpool.tile([P, UNROLL, 2, BC], dt.uint8,
                                      tag="mwin")
                    nc.vector.tensor_copy(out=mwin[:], in_=mw)
                    # one block-wide identity matmul injects xp for all
                    # UNROLL steps (start=True on the whole bank)
                    nc.tensor.matmul(
                        out=zsblk[:].rearrange("p u g d b -> p (u g d b)"),
                        lhsT=idb[:],
                        rhs=xstage[:].rearrange("p u g d b -> p (u g d b)"),
                        start=True, stop=False, skip_group_check=True)
                    for j in range(UNROLL):
                        jp = j - 1 if j > 0 else UNROLL - 1
                        for dd, wh in ((0, whf), (1, whb)):
                            for g in range(4):
                                nc.tensor.matmul(
                                    out=zsblk[:, j, g, dd, :],
                                    lhsT=wh[:, g * H:(g + 1) * H],
                                    rhs=hwin[:, jp, dd, :],
                                    start=False, stop=(dd == 1 and g == 3),
                                    skip_group_check=True)
                        tall = wpool.tile([P, 4, 2, BC], dt.float32, tag="tall")
                        nc.scalar.activation(
                            out=tall[:], in_=zsblk[:, j, :, :, :],
                            func=AF.Tanh, scale=0.5)
                        wt = wpool.tile([P, 2, BC], dt.float32, tag="wt")
                        nc.vector.scalar_tensor_tensor(
                            out=wt[:], in0=tall[:, 0, :, :], scalar=1.0,
                            in1=tall[:, 2, :, :], op0=ALU.add, op1=ALU.mult)
                        pt_ = wpool.tile([P, 2, BC], dt.float32, tag="pt")
                        nc.vector.scalar_tensor_tensor(
                            out=pt_[:], in0=tall[:, 1, :, :], scalar=1.0,
                            in1=Cs[:], op0=ALU.add, op1=ALU.mult)
                        nc.vector.scalar_tensor_tensor(
                            out=Cs[:], in0=pt_[:], scalar=0.5,
                            in1=wt[:], op0=ALU.mult, op1=ALU.add)
                        tct = wpool.tile([P, 2, BC], dt.float32, tag="tct")
                        nc.scalar.activation(
                            out=tct[:], in_=Cs[:], func=AF.Tanh, scale=0.5)
                        rt = wpool.tile([P, 2, BC], dt.bfloat16, tag="rt")
                        nc.vector.scalar_tensor_tensor(
                            out=rt[:], in0=tall[:, 3, :, :], scalar=1.0,
                            in1=tct[:], op0=ALU.add, op1=ALU.mult)
                        # masked lanes carry h_{t-1}: seed slot j with the
                        # previous h, then overwrite unmasked lanes
                        nc.scalar.copy(out=hwin[:, j, :, :],
                                       in_=hwin[:, jp, :, :])
                        nc.vector.copy_predicated(
                            out=hwin[:, j, :, :],
                            mask=mwin[:, j, :, :], data=rt[:])
                    nc.scalar.copy(
                        out=yw, in_=hwin[:].rearrange("p u d b -> p d u b"))

            scan_layer(0, y0)

            # ---- layer-1 xp from SBUF y0 (reversed views for time flips)
            # dir f at fwd t: y0f straight, y0b slice reversed
            # dir b at scan s: y0f slice reversed, y0b straight
            y0r = y0[:, :, ::-1, :]
            for dd, d in ((0, "f"), (1, "b")):
                for n in range(NXC):
                    t0, t1 = n * TCH, (n + 1) * TCH
                    if dd == 0:
                        rf = y0[:, 0, t0:t1, :]
                        rb = y0r[:, 1, t0:t1, :]
                    else:
                        rf = y0r[:, 0, t0:t1, :]
                        rb = y0[:, 1, t0:t1, :]
                    for g in range(4):
                        ps = psx.tile([P, TCH, BC], dt.float32, tag="psxp")
                        nc.tensor.matmul(
                            out=ps[:],
                            lhsT=wsb[f"wx1{d}f"][:, g * H:(g + 1) * H],
                            rhs=rf, start=True, stop=False)
                        nc.tensor.matmul(
                            out=ps[:],
                            lhsT=wsb[f"wx1{d}b"][:, g * H:(g + 1) * H],
                            rhs=rb, start=False, stop=True)
                        xp_epilogue(1, dd, d, n, g, ps)

            scan_layer(1, y1)

            # ---- pooling: PE-transpose [h -> token] chunks, reduce over 2H
            with tc.tile_pool(name="ep", bufs=4) as epool:
                fmxf = cpool.tile([P, NCH], dt.float32, tag="fmxf")
                fsmf = cpool.tile([P, NCH], dt.float32, tag="fsmf")
                fmxb = cpool.tile([P, NCH], dt.float32, tag="fmxb")
                fsmb = cpool.tile([P, NCH], dt.float32, tag="fsmb")
                for c in range(NCH):
                    tp = psx.tile([P, 2, P], dt.float32, tag="psxp")
                    for dd in (0, 1):
                        nc.tensor.transpose(
                            out=tp[:, dd, :],
                            in_=y1[:, dd, c * TPC:(c + 1) * TPC, :],
                            identity=ident[:])
                    # backward dir is scan-order: chunk c is fwd chunk 31-c
                    cb = NCH - 1 - c
                    nc.vector.tensor_reduce(
                        out=fmxf[:, c:c + 1], in_=tp[:, 0, :],
                        axis=mybir.AxisListType.XYZW, op=ALU.max)
                    nc.vector.tensor_reduce(
                        out=fsmf[:, c:c + 1], in_=tp[:, 0, :],
                        axis=mybir.AxisListType.XYZW, op=ALU.add)
                    nc.vector.tensor_reduce(
                        out=fmxb[:, cb:cb + 1], in_=tp[:, 1, :],
                        axis=mybir.AxisListType.XYZW, op=ALU.max)
                    nc.vector.tensor_reduce(
                        out=fsmb[:, cb:cb + 1], in_=tp[:, 1, :],
                        axis=mybir.AxisListType.XYZW, op=ALU.add)
                # permute backward-dir partitions to forward token order
                pmx = psf.tile([P, 2, NCH], dt.float32, tag="pfeat")
                nc.tensor.matmul(out=pmx[:, 0, :], lhsT=perm[:], rhs=fmxb[:],
                                 start=True, stop=True)
                nc.tensor.matmul(out=pmx[:, 1, :], lhsT=perm[:], rhs=fsmb[:],
                                 start=True, stop=True)
                fmx = epool.tile([P, NCH], dt.float32, tag="fmx")
                nc.vector.tensor_tensor(
                    out=fmx[:], in0=fmxf[:], in1=pmx[:, 0, :], op=ALU.max)
                fsum = epool.tile([P, NCH], dt.float32, tag="fsum")
                nc.vector.tensor_tensor(
                    out=fsum[:], in0=fsmf[:], in1=pmx[:, 1, :], op=ALU.add)
                featv = feat_dram.rearrange("s t b -> s (t b)")
                nc.sync.dma_start(
                    out=featv[0].rearrange("(c p) -> p c", p=P), in_=fmx[:])
                nc.sync.dma_start(
                    out=featv[1].rearrange("(c p) -> p c", p=P), in_=fsum[:])

                # ---- FC head: out = relu(featT.T @ fcw + b)
                pfc = psf.tile([BC, NCLS], dt.float32, tag="pfc")
                NQ = 2 * T // P
                for q in range(NQ):
                    lq = epool.tile([P, BC], dt.float32, tag="lq")
                    pool_i, tq = divmod(q * P, T)
                    nc.sync.dma_start(
                        out=lq[:], in_=feat_dram[pool_i, tq:tq + P, :])
                    nc.tensor.matmul(
                        out=pfc[:], lhsT=lq[:], rhs=fcw_t[:, q, :],
                        start=(q == 0), stop=(q == NQ - 1))
                ob = epool.tile([BC, NCLS], dt.float32, tag="ob")
                nc.vector.tensor_tensor(
                    out=ob[:], in0=pfc[:], in1=fcb_t[:], op=ALU.add)
                nc.vector.tensor_scalar(
                    out=ob[:], in0=ob[:], scalar1=0.0, scalar2=None,
                    op0=ALU.max)
                nc.sync.dma_start(out=out_d[:], in_=ob[:])

    split_multi_waits(nc)
    return nc


_cached_nc = None


def _get_nc():
    global _cached_nc
    if _cached_nc is None:
        _install_hook()
        _cached_nc = _build()
    return _cached_nc


def _in_maps(inputs):
    w = _fold_weights(inputs)
    x = np.asarray(inputs["x"]).astype(np.int32)  # [64, 512]
    shared = {
        "emb": w["emb"], "ident": w["ident"], "idb": w["idb"],
        "perm": w["perm"], "fcw": w["fcw"], "fcb_rep": w["fcb_rep"],
    }
    for l in (0, 1):
        for d in ("f", "b"):
            shared[f"wh{l}{d}"] = w[f"wh{l}{d}"]
            shared[f"bcol{l}{d}"] = w[f"bcol{l}{d}"]
            if l == 0:
                shared[f"wx0{d}"] = w[f"wx0{d}"]
            else:
                shared[f"wx1{d}f"] = w[f"wx1{d}f"]
                shared[f"wx1{d}b"] = w[f"wx1{d}b"]
    maps = []
    for c in range(NCORES):
        xc = x[c * BC:(c + 1) * BC]            # [BC, T]
        idx = np.ascontiguousarray(xc.T).reshape(-1).astype(np.int32)
        m = (xc != 0).astype(np.uint8).T       # [T, BC]
        m2 = np.stack([m, m[::-1]], axis=1)    # [T, 2, BC]
        m2 = np.broadcast_to(m2[None], (P, T, 2, BC))
        maps.append(dict(shared, idx=idx, m2=np.ascontiguousarray(m2)))
    return maps


def _run(inputs, trace=False):
    from concourse.bass_utils import run_bass_kernel_spmd
    nc = _get_nc()
    maps = _in_maps(inputs)
    res = run_bass_kernel_spmd(nc, maps, list(range(NCORES)), trace=trace)
    out = np.concatenate([res.results[c]["out"] for c in range(NCORES)], axis=0)
    return out.astype(np.float32), res


def kernel(**inputs):
    out, _ = _run(inputs, trace=False)
    return out


def run_traced(inputs):
    out, res = _run(inputs, trace=True)
    return out, res


# revision 14
# speedup vs baseline: 1.2000x; 1.2000x over previous
"""Bass/TRN2 kernel for nn_BiRNNLayers: 2-layer BiLSTM (B=64, T=512, H=128,
vocab 50000) with masked Keras-style scan, feature pooling and FC head.

Strategy (8 NeuronCores, data-parallel over batch, 8 rows/core):
- Embedding gather on device (indirect DMA), PE transpose to H-on-partitions.
- Merged-direction scan: one [128, 4(gate), 2(dir), BC] tile per step; the
  xp term is injected into PSUM by an identity matmul (start=True) and the 8
  per-gate recurrent matmuls accumulate on top (start=False) -> the vector
  add drops off the critical chain.
- All activations share one (Tanh, scale=0.5) signature so the scalar engine
  never reloads its activation table; sigmoid gates are tanh(z/2) with the
  column scaling folded into weights; state kept as H'=2h, C'=2c.
- bf16 weights/hidden state (4x cheaper LDWEIGHTS + matmul); cell math fp32.
- Masked carry: C-carry exact via gate saturation (+-KSAT folded into xp),
  H-carry via copy_predicated with a u8 mask.
- Everything lives in SBUF (xp, y0, y1); no per-step DMA. The backward
  direction is stored scan-order; time reversal is done with reversed SBUF
  views (matmul rhs) and a 128x128 permutation matmul in the pooling stage.
"""
import numpy as np
import ml_dtypes

import concourse.bass as bass
import concourse.mybir as mybir
import concourse.tile as tile
import bass_rust

P = 128
T = 512
H = 128
E = 128
B_FULL = 64
NCORES = 8
BC = B_FULL // NCORES  # batch rows per core
VOCAB = 50000
NCLS = 10
KSAT = 40.0            # xp saturation offset for masked steps (tanh arg +-20)
UNROLL = 8

AF = mybir.ActivationFunctionType
ALU = mybir.AluOpType
dt = mybir.dt
bf16 = ml_dtypes.bfloat16

_hook_installed = False


def _install_hook():
    """Surface compile-hook tracebacks (PJRT swallows them otherwise)."""
    global _hook_installed
    if _hook_installed:
        return
    _hook_installed = True
    import traceback
    import concourse.bass2jax as bass2jax
    import libneuronxla

    orig = bass2jax.neuronx_cc_hook

    def dbg_hook(*a, **k):
        try:
            return orig(*a, **k)
        except BaseException:
            traceback.print_exc()
            raise

    bass2jax.neuronx_cc_hook = dbg_hook
    if not hasattr(libneuronxla, "orig_neuronx_cc"):
        libneuronxla.orig_neuronx_cc = libneuronxla.neuronx_cc
    libneuronxla.neuronx_cc = dbg_hook


def split_multi_waits(nc):
    """This container's walrus encodes at most one sem wait per instruction;
    hoist extra waits onto preceding same-engine NoOps."""
    for fn in nc.m.functions:
        for bb in fn.blocks:
            out = []
            changed = False
            for inst in bb.instructions:
                si = inst.sync_info
                waits = list(si.on_wait) if si is not None and si.on_wait else []
                if len(waits) > 1:
                    changed = True
                    for k, w in enumerate(waits[:-1]):
                        nop = mybir.InstNoOp(name=f"{inst.name}-sw{k}")
                        nop.engine = inst.engine
                        nop.sync_info = bass_rust.SyncInfo(on_wait=[w], on_update=[])
                        out.append(nop)
                    inst.sync_info = bass_rust.SyncInfo(
                        on_wait=[waits[-1]], on_update=list(si.on_update)
                    )
                out.append(inst)
            if changed:
                bb.instructions = out


# ---------------------------------------------------------------------------
# host-side weight folding
# ---------------------------------------------------------------------------

def _fold_weights(inputs):
    f32 = np.float32
    # all activations are tanh(0.5*stored): sigmoid gates (i,f,o) need
    # stored = z (cs=1), the g gate needs stored = 2*z (cs=2)
    cs2 = np.concatenate([
        np.ones(H), np.ones(H), np.full(H, 2.0), np.ones(H)
    ]).astype(f32)

    w = {}
    for l in (0, 1):
        for d in ("f", "b"):
            Wx = np.asarray(inputs[f"Wx_{d}{l}"], f32)
            Wh = np.asarray(inputs[f"Wh_{d}{l}"], f32)
            b = np.asarray(inputs[f"b_{d}{l}"], f32)
            w[f"wh{l}{d}"] = ((Wh * 0.5) * cs2).astype(bf16)
            be = (b * cs2).astype(f32)
            w[f"bcol{l}{d}"] = np.ascontiguousarray(
                be.reshape(4, H).T)  # [128, 4]
            if l == 0:
                w[f"wx0{d}"] = (Wx * cs2).astype(bf16)
            else:
                # rows 0:128 multiply y0f = 2*hf, rows 128:256 multiply y0b
                w[f"wx1{d}f"] = ((Wx[0:H] * 0.5) * cs2).astype(bf16)
                w[f"wx1{d}b"] = ((Wx[H:2 * H] * 0.5) * cs2).astype(bf16)

    w["emb"] = np.asarray(inputs["emb"], f32)

    fcw = np.asarray(inputs["fc_W"], f32).copy()  # [2T, 10]
    fcw[:T] *= 0.5          # mx rows: feat carries 2*mx
    fcw[T:] *= 1.0 / 512.0  # av rows: feat carries sum(2h) over 256 feats
    w["fcw"] = fcw.astype(f32)
    w["fcb_rep"] = np.tile(np.asarray(inputs["fc_b"], f32)[None, :], (BC, 1))
    w["ident"] = np.eye(P, dtype=f32)
    w["idb"] = np.eye(P, dtype=f32).astype(bf16)
    # pooling-time partition permutation for the scan-order backward dir:
    # token partition p=(t_local*8+b) maps to p'=(15-t_local)*8+b
    pm = np.zeros((P, P), dtype=f32)
    for k in range(P):
        pm[k, (15 - k // BC) * BC + k % BC] = 1.0
    w["perm"] = pm
    return w


# ---------------------------------------------------------------------------
# device program
# ---------------------------------------------------------------------------

def _build():
    nc = bass.Bass("TRN2", target_bir_lowering=False, debug=False,
                   num_devices=NCORES)

    def di(name, shape, dtype=dt.float32):
        return nc.dram_tensor(name, shape, dtype, kind="ExternalInput")

    emb_d = di("emb", [VOCAB + 1, E])
    ident_d = di("ident", [P, P])
    idb_d = di("idb", [P, P], dt.bfloat16)
    perm_d = di("perm", [P, P])
    idx_d = di("idx", [T * BC], dt.int32)
    m2_d = di("m2", [P, T, 2, BC], dt.uint8)
    fcw_d = di("fcw", [2 * T, NCLS])
    fcb_d = di("fcb_rep", [BC, NCLS])
    wdram = {}
    for l in (0, 1):
        for d in ("f", "b"):
            wdram[f"wh{l}{d}"] = di(f"wh{l}{d}", [H, 4 * H], dt.bfloat16)
            wdram[f"bcol{l}{d}"] = di(f"bcol{l}{d}", [P, 4])
            if l == 0:
                wdram[f"wx0{d}"] = di(f"wx0{d}", [E, 4 * H], dt.bfloat16)
            else:
                wdram[f"wx1{d}f"] = di(f"wx1{d}f", [H, 4 * H], dt.bfloat16)
                wdram[f"wx1{d}b"] = di(f"wx1{d}b", [H, 4 * H], dt.bfloat16)

    out_d = nc.dram_tensor("out", [BC, NCLS], dt.float32, kind="ExternalOutput")
    feat_dram = nc.dram_tensor("feat", [2, T, BC], dt.float32)

    NTOK = T * BC            # 4096 tokens per core
    NCH = NTOK // P          # 32 gather/pool chunks
    NXC = NTOK // 512        # 8 xp matmul chunks
    TCH = 512 // BC          # 64 timesteps per xp chunk
    TPC = P // BC            # 16 timesteps per pooling chunk

    with tile.TileContext(nc) as tc:
        with (
            tc.tile_pool(name="const", bufs=1) as cpool,
            tc.tile_pool(name="work", bufs=4) as wpool,
            tc.tile_pool(name="psx", bufs=2, space="PSUM") as psx,
            tc.tile_pool(name="psz", bufs=4, space="PSUM") as psz,
            tc.tile_pool(name="psf", bufs=1, space="PSUM") as psf,
        ):
            # ---- constant loads
            ident = cpool.tile([P, P], dt.float32, tag="ident")
            nc.sync.dma_start(out=ident[:], in_=ident_d[:])
            idb = cpool.tile([P, P], dt.bfloat16, tag="idb")
            nc.sync.dma_start(out=idb[:], in_=idb_d[:])
            perm = cpool.tile([P, P], dt.float32, tag="perm")
            nc.sync.dma_start(out=perm[:], in_=perm_d[:])
            idx_t = cpool.tile([P, NCH], dt.int32, tag="idx")
            nc.sync.dma_start(
                out=idx_t[:], in_=idx_d.rearrange("(c p) -> p c", p=P))
            m2 = cpool.tile([P, T, 2, BC], dt.uint8, tag="m2", name="m2")
            nc.sync.dma_start(out=m2[:], in_=m2_d[:])
            wsb = {}
            for k, dr in wdram.items():
                sh = list(dr.shape)
                wt_ = cpool.tile(sh, dr.dtype, tag=k, name=k)
                nc.sync.dma_start(out=wt_[:], in_=dr[:])
                wsb[k] = wt_
            fcw_t = cpool.tile([P, 2 * T // P, NCLS], dt.float32, tag="fcw")
            nc.sync.dma_start(
                out=fcw_t[:], in_=fcw_d.rearrange("(q p) c -> p q c", p=P))
            fcb_t = cpool.tile([BC, NCLS], dt.float32, tag="fcb")
            nc.sync.dma_start(out=fcb_t[:], in_=fcb_d[:])

            # persistent SBUF state
            xpT = cpool.tile([P, T, 4, 2, BC], dt.bfloat16, tag="xpT",
                             name="xpT")
            y0 = cpool.tile([P, 2, T, BC], dt.bfloat16, tag="y0", name="y0")
            y1 = cpool.tile([P, 2, T, BC], dt.float32, tag="y1", name="y1")
            # rolling H history for one UNROLL block; slot j = h after step j
            hwin = cpool.tile([P, UNROLL, 2, BC], dt.bfloat16, tag="hwin",
                              name="hwin")
            Cs = cpool.tile([P, 2, BC], dt.float32, tag="Cs", name="Cs")

            def xp_epilogue(l, dd, d, n, g, ps):
                """xpT[:, chunk, g, dd, :] = ps + bias_col + K_g*(1-m)."""
                t0, t1 = n * TCH, (n + 1) * TCH
                dst = xpT[:, t0:t1, g, dd, :]
                bcol = wsb[f"bcol{l}{d}"]
                kg = -KSAT if g == 0 else (KSAT if g == 1 else 0.0)
                if kg != 0.0:
                    # tmp = m*(-kg) + ps in fp32; the big +-KSAT intermediate
                    # must not round through bf16 (ulp(40) = 0.25), so only
                    # the final small-valued sum is written to the bf16 dst
                    tmp = wpool.tile([P, TCH, BC], dt.float32, tag="xptmp")
                    nc.vector.scalar_tensor_tensor(
                        out=tmp[:], in0=m2[:, t0:t1, dd, :], scalar=-kg,
                        in1=ps[:], op0=ALU.mult, op1=ALU.add)
                    nc.vector.tensor_scalar(
                        out=dst, in0=tmp[:], scalar1=bcol[:, g:g + 1],
                        scalar2=float(kg), op0=ALU.add, op1=ALU.add)
                else:
                    nc.vector.tensor_scalar(
                        out=dst, in0=ps[:], scalar1=bcol[:, g:g + 1],
                        scalar2=None, op0=ALU.add)

            # ---- embedding gather + transpose + layer-0 xp
            with tc.tile_pool(name="gph", bufs=3) as gpool, \
                 tc.tile_pool(name="gbig", bufs=1) as gbig:
                g128 = gbig.tile([P, T, BC], dt.bfloat16, tag="g128")
                g128f = g128[:].rearrange("p t b -> p (t b)")
                for c in range(NCH):
                    gr = gpool.tile([P, E], dt.float32, tag="gr")
                    nc.gpsimd.indirect_dma_start(
                        out=gr[:], out_offset=None, in_=emb_d[:],
                        in_offset=bass.IndirectOffsetOnAxis(
                            ap=idx_t[:, c:c + 1], axis=0),
                    )
                    pt = psx.tile([P, P], dt.float32, tag="psxp")
                    nc.tensor.transpose(out=pt[:], in_=gr[:], identity=ident[:])
                    nc.vector.tensor_copy(
                        out=g128f[:, c * P:(c + 1) * P], in_=pt[:])

                for dd, d, rv in ((0, "f", g128[:]), (1, "b", g128[:, ::-1, :])):
                    wxa = wsb[f"wx0{d}"]
                    for n in range(NXC):
                        t0, t1 = n * TCH, (n + 1) * TCH
                        for g in range(4):
                            ps = psx.tile([P, TCH, BC], dt.float32, tag="psxp")
                            nc.tensor.matmul(
                                out=ps[:], lhsT=wxa[:, g * H:(g + 1) * H],
                                rhs=rv[:, t0:t1, :], start=True, stop=True)
                            xp_epilogue(0, dd, d, n, g, ps)

            # ---- the merged-direction scan
            def scan_layer(l, ysb):
                nc.vector.memset(hwin[:], 0.0)
                nc.vector.memset(Cs[:], 0.0)
                whf = wsb[f"wh{l}f"]
                whb = wsb[f"wh{l}b"]
                with tc.For_i(0, T, UNROLL) as t0:
                    # one dynamic AP per engine per block (per-step ds()
                    # offsets exhaust the register file); everything inside
                    # the block indexes statically
                    xw = xpT[:, bass.ds(t0, UNROLL), :, :, :]
                    mw = m2[:, bass.ds(t0, UNROLL), :, :]
                    yw = ysb[:, :, bass.ds(t0, UNROLL), :]
                    # stage the block's xp in SBUF (only the DVE handles
                    # dynamic APs); each step injects it into PSUM via an
                    # identity matmul so the accumulation group is pure-PE
                    # (mixed DVE-write + PE-accumulate on a PSUM bank races)
                    xstage = wpool.tile([P, UNROLL, 4, 2, BC], dt.bfloat16,
                                        tag="xstage")
                    nc.vector.tensor_copy(out=xstage[:], in_=xw)
                    zsblk = psz.tile([P, UNROLL, 4, 2, BC], dt.float32,
                                     tag="zs")
                    mwin = wpool.tile([P, UNROLL, 2, BC], dt.uint8,
                                      tag="mwin")
                    nc.vector.tensor_copy(out=mwin[:], in_=mw)
                    # one block-wide identity matmul injects xp for all
                    # UNROLL steps (start=True on the whole bank)
                    nc.tensor.matmul(
                        out=zsblk[:].rearrange("p u g d b -> p (u g d b)"),
                        lhsT=idb[:],
                        rhs=xstage[:].rearrange("p u g d b -> p (u g d b)"),
                        start=True, stop=False, skip_group_check=True)
                    for j in range(UNROLL):
                        jp = j - 1 if j > 0 else UNROLL - 1
                        for dd, wh in ((0, whf), (1, whb)):
                            for g in range(4):
                                nc.tensor.matmul(
                                    out=zsblk[:, j, g, dd, :],
                                    lhsT=wh[:, g * H:(g + 1) * H],
                                    rhs=hwin[:, jp, dd, :],
                                    start=False, stop=(dd == 1 and g == 3),
                                    skip_group_check=True)
                        tall = wpool.tile([P, 4, 2, BC], dt.float32, tag="tall")
                        nc.scalar.activation(
                            out=tall[:], in_=zsblk[:, j, :, :, :],
                            func=AF.Tanh, scale=0.5)
                        wt = wpool.tile([P, 2, BC], dt.float32, tag="wt")
                        nc.vector.scalar_tensor_tensor(
                            out=wt[:], in0=tall[:, 0, :, :], scalar=1.0,
                            in1=tall[:, 2, :, :], op0=ALU.add, op1=ALU.mult)
                        pt_ = wpool.tile([P, 2, BC], dt.float32, tag="pt")
                        nc.vector.scalar_tensor_tensor(
                            out=pt_[:], in0=tall[:, 1, :, :], scalar=1.0,
                            in1=Cs[:], op0=ALU.add, op1=ALU.mult)
                        nc.vector.scalar_tensor_tensor(
                            out=Cs[:], in0=pt_[:], scalar=0.5,
                            in1=wt[:], op0=ALU.mult, op1=ALU.add)
                        tct = wpool.tile([P, 2, BC], dt.float32, tag="tct")
                        nc.scalar.activation(
                            out=tct[:], in_=Cs[:], func=AF.Tanh, scale=0.5)
                        rt = wpool.tile([P, 2, BC], dt.bfloat16, tag="rt")
                        nc.vector.scalar_tensor_tensor(
                            out=rt[:], in0=tall[:, 3, :, :], scalar=1.0,
                            in1=tct[:], op0=ALU.add, op1=ALU.mult)
                        # masked lanes carry h_{t-1}: seed slot j with the
                        # previous h, then overwrite unmasked lanes
                        nc.scalar.copy(out=hwin[:, j, :, :],
                                       in_=hwin[:, jp, :, :])
                        nc.vector.copy_predicated(
                            out=hwin[:, j, :, :],
                            mask=mwin[:, j, :, :], data=rt[:])
                    nc.scalar.copy(
                        out=yw, in_=hwin[:].rearrange("p u d b -> p d u b"))

            scan_layer(0, y0)

            # ---- layer-1 xp from SBUF y0 (reversed views for time flips)
            # dir f at fwd t: y0f straight, y0b slice reversed
            # dir b at scan s: y0f slice reversed, y0b straight
            y0r = y0[:, :, ::-1, :]
            for dd, d in ((0, "f"), (1, "b")):
                for n in range(NXC):
                    t0, t1 = n * TCH, (n + 1) * TCH
                    if dd == 0:
                        rf = y0[:, 0, t0:t1, :]
                        rb = y0r[:, 1, t0:t1, :]
                    else:
                        rf = y0r[:, 0, t0:t1, :]
                        rb = y0[:, 1, t0:t1, :]
                    for g in range(4):
                        ps = psx.tile([P, TCH, BC], dt.float32, tag="psxp")
                        nc.tensor.matmul(
                            out=ps[:],
                            lhsT=wsb[f"wx1{d}f"][:, g * H:(g + 1) * H],
                            rhs=rf, start=True, stop=False)
                        nc.tensor.matmul(
                            out=ps[:],
                            lhsT=wsb[f"wx1{d}b"][:, g * H:(g + 1) * H],
                            rhs=rb, start=False, stop=True)
                        xp_epilogue(1, dd, d, n, g, ps)

            scan_layer(1, y1)

            # ---- pooling: PE-transpose [h -> token] chunks, reduce over 2H
            with tc.tile_pool(name="ep", bufs=4) as epool:
                fmxf = cpool.tile([P, NCH], dt.float32, tag="fmxf")
                fsmf = cpool.tile([P, NCH], dt.float32, tag="fsmf")
                fmxb = cpool.tile([P, NCH], dt.float32, tag="fmxb")
                fsmb = cpool.tile([P, NCH], dt.float32, tag="fsmb")
                for c in range(NCH):
                    tp = psx.tile([P, 2, P], dt.float32, tag="psxp")
                    for dd in (0, 1):
                        nc.tensor.transpose(
                            out=tp[:, dd, :],
                            in_=y1[:, dd, c * TPC:(c + 1) * TPC, :],
                            identity=ident[:])
                    # backward dir is scan-order: chunk c is fwd chunk 31-c
                    cb = NCH - 1 - c
                    nc.vector.tensor_reduce(
                        out=fmxf[:, c:c + 1], in_=tp[:, 0, :],
                        axis=mybir.AxisListType.XYZW, op=ALU.max)
                    nc.vector.tensor_reduce(
                        out=fsmf[:, c:c + 1], in_=tp[:, 0, :],
                        axis=mybir.AxisListType.XYZW, op=ALU.add)
                    nc.vector.tensor_reduce(
                        out=fmxb[:, cb:cb + 1], in_=tp[:, 1, :],
                        axis=mybir.AxisListType.XYZW, op=ALU.max)
                    nc.vector.tensor_reduce(
                        out=fsmb[:, cb:cb + 1], in_=tp[:, 1, :],
                        axis=mybir.AxisListType.XYZW, op=ALU.add)
                # permute backward-dir partitions to forward token order
                pmx = psf.tile([P, 2, NCH], dt.float32, tag="pfeat")
                nc.tensor.matmul(out=pmx[:, 0, :], lhsT=perm[:], rhs=fmxb[:],
                                 start=True, stop=True)
                nc.tensor.matmul(out=pmx[:, 1, :], lhsT=perm[:], rhs=fsmb[:],
                                 start=True, stop=True)
                fmx = epool.tile([P, NCH], dt.float32, tag="fmx")
                nc.vector.tensor_tensor(
                    out=fmx[:], in0=fmxf[:], in1=pmx[:, 0, :], op=ALU.max)
                fsum = epool.tile([P, NCH], dt.float32, tag="fsum")
                nc.vector.tensor_tensor(
                    out=fsum[:], in0=fsmf[:], in1=pmx[:, 1, :], op=ALU.add)
                featv = feat_dram.rearrange("s t b -> s (t b)")
                nc.sync.dma_start(
                    out=featv[0].rearrange("(c p) -> p c", p=P), in_=fmx[:])
                nc.sync.dma_start(
                    out=featv[1].rearrange("(c p) -> p c", p=P), in_=fsum[:])

                # ---- FC head: out = relu(featT.T @ fcw + b)
                pfc = psf.tile([BC, NCLS], dt.float32, tag="pfc")
                NQ = 2 * T // P
                for q in range(NQ):
                    lq = epool.tile([P, BC], dt.float32, tag="lq")
                    pool_i, tq = divmod(q * P, T)
                    nc.sync.dma_start(
                        out=lq[:], in_=feat_dram[pool_i, tq:tq + P, :])
                    nc.tensor.matmul(
                        out=pfc[:], lhsT=lq[:], rhs=fcw_t[:, q, :],
                        start=(q == 0), stop=(q == NQ - 1))
                ob = epool.tile([BC, NCLS], dt.float32, tag="ob")
                nc.vector.tensor_tensor(
                    out=ob[:], in0=pfc[:], in1=fcb_t[:], op=ALU.add)
                nc.vector.tensor_scalar(
                    out=ob[:], in0=ob[:], scalar1=0.0, scalar2=None,
                    op0=ALU.max)
                nc.sync.dma_start(out=out_d[:], in_=ob[:])

    split_multi_waits(nc)
    return nc


_cached_nc = None


def _get_nc():
    global _cached_nc
    if _cached_nc is None:
        _install_hook()
        _cached_nc = _build()
    return _cached_nc


def _in_maps(inputs):
    w = _fold_weights(inputs)
    x = np.asarray(inputs["x"]).astype(np.int32)  # [64, 512]
    shared = {
        "emb": w["emb"], "ident": w["ident"], "idb": w["idb"],
        "perm": w["perm"], "fcw": w["fcw"], "fcb_rep": w["fcb_rep"],
    }
    for l in (0, 1):
        for d in ("f", "b"):
            shared[f"wh{l}{d}"] = w[f"wh{l}{d}"]
            shared[f"bcol{l}{d}"] = w[f"bcol{l}{d}"]
            if l == 0:
                shared[f"wx0{d}"] = w[f"wx0{d}"]
            else:
                shared[f"wx1{d}f"] = w[f"wx1{d}f"]
                shared[f"wx1{d}b"] = w[f"wx1{d}b"]
    maps = []
    for c in range(NCORES):
        xc = x[c * BC:(c + 1) * BC]            # [BC, T]
        idx = np.ascontiguousarray(xc.T).reshape(-1).astype(np.int32)
        m = (xc != 0).astype(np.uint8).T       # [T, BC]
        m2 = np.stack([m, m[::-1]], axis=1)    # [T, 2, BC]
        m2 = np.broadcast_to(m2[None], (P, T, 2, BC))
        maps.append(dict(shared, idx=idx, m2=np.ascontiguousarray(m2)))
    return maps


def _run(inputs, trace=False):
    from concourse.bass_utils import run_bass_kernel_spmd
    nc = _get_nc()
    maps = _in_maps(inputs)
    res = run_bass_kernel_spmd(nc, maps, list(range(NCORES)), trace=trace)
    out = np.concatenate([res.results[c]["out"] for c in range(NCORES)], axis=0)
    return out.astype(np.float32), res


def kernel(**inputs):
    out, _ = _run(inputs, trace=False)
    return out


def run_traced(inputs):
    out, res = _run(inputs, trace=True)
    return out, res


# revision 17
# speedup vs baseline: 1.2621x; 1.0517x over previous
"""Bass/TRN2 kernel for nn_BiRNNLayers: 2-layer BiLSTM (B=64, T=512, H=128,
vocab 50000) with masked Keras-style scan, feature pooling and FC head.

Strategy (8 NeuronCores, data-parallel over batch, 8 rows/core):
- Embedding gather on device (indirect DMA), PE transpose to H-on-partitions.
- Merged-direction scan: one [128, 4(gate), 2(dir), BC] tile per step; the
  xp term is injected into PSUM by an identity matmul (start=True) and the 8
  per-gate recurrent matmuls accumulate on top (start=False) -> the vector
  add drops off the critical chain.
- All activations share one (Tanh, scale=0.5) signature so the scalar engine
  never reloads its activation table; sigmoid gates are tanh(z/2) with the
  column scaling folded into weights; state kept as H'=2h, C'=2c.
- bf16 weights/hidden state (4x cheaper LDWEIGHTS + matmul); cell math fp32.
- Masked carry: C-carry exact via gate saturation (+-KSAT folded into xp),
  H-carry via copy_predicated with a u8 mask.
- Everything lives in SBUF (xp, y0, y1); no per-step DMA. The backward
  direction is stored scan-order; time reversal is done with reversed SBUF
  views (matmul rhs) and a 128x128 permutation matmul in the pooling stage.
"""
import numpy as np
import ml_dtypes

import concourse.bass as bass
import concourse.mybir as mybir
import concourse.tile as tile
import bass_rust

P = 128
T = 512
H = 128
E = 128
B_FULL = 64
NCORES = 8
BC = B_FULL // NCORES  # batch rows per core
VOCAB = 50000
NCLS = 10
KSAT = 40.0            # xp saturation offset for masked steps (tanh arg +-20)
UNROLL = 16

AF = mybir.ActivationFunctionType
ALU = mybir.AluOpType
dt = mybir.dt
bf16 = ml_dtypes.bfloat16

_hook_installed = False


def _install_hook():
    """Surface compile-hook tracebacks (PJRT swallows them otherwise)."""
    global _hook_installed
    if _hook_installed:
        return
    _hook_installed = True
    import traceback
    import concourse.bass2jax as bass2jax
    import libneuronxla

    orig = bass2jax.neuronx_cc_hook

    def dbg_hook(*a, **k):
        try:
            return orig(*a, **k)
        except BaseException:
            traceback.print_exc()
            raise

    bass2jax.neuronx_cc_hook = dbg_hook
    if not hasattr(libneuronxla, "orig_neuronx_cc"):
        libneuronxla.orig_neuronx_cc = libneuronxla.neuronx_cc
    libneuronxla.neuronx_cc = dbg_hook


def split_multi_waits(nc):
    """This container's walrus encodes at most one sem wait per instruction;
    hoist extra waits onto preceding same-engine NoOps."""
    for fn in nc.m.functions:
        for bb in fn.blocks:
            out = []
            changed = False
            for inst in bb.instructions:
                si = inst.sync_info
                waits = list(si.on_wait) if si is not None and si.on_wait else []
                if len(waits) > 1:
                    changed = True
                    for k, w in enumerate(waits[:-1]):
                        nop = mybir.InstNoOp(name=f"{inst.name}-sw{k}")
                        nop.engine = inst.engine
                        nop.sync_info = bass_rust.SyncInfo(on_wait=[w], on_update=[])
                        out.append(nop)
                    inst.sync_info = bass_rust.SyncInfo(
                        on_wait=[waits[-1]], on_update=list(si.on_update)
                    )
                out.append(inst)
            if changed:
                bb.instructions = out


# ---------------------------------------------------------------------------
# host-side weight folding
# ---------------------------------------------------------------------------

def _fold_weights(inputs):
    f32 = np.float32
    # all activations are tanh(0.5*stored): sigmoid gates (i,f,o) need
    # stored = z (cs=1), the g gate needs stored = 2*z (cs=2)
    cs2 = np.concatenate([
        np.ones(H), np.ones(H), np.full(H, 2.0), np.ones(H)
    ]).astype(f32)

    w = {}
    for l in (0, 1):
        for d in ("f", "b"):
            Wx = np.asarray(inputs[f"Wx_{d}{l}"], f32)
            Wh = np.asarray(inputs[f"Wh_{d}{l}"], f32)
            b = np.asarray(inputs[f"b_{d}{l}"], f32)
            w[f"wh{l}{d}"] = ((Wh * 0.5) * cs2).astype(bf16)
            be = (b * cs2).astype(f32)
            w[f"bcol{l}{d}"] = np.ascontiguousarray(
                be.reshape(4, H).T)  # [128, 4]
            if l == 0:
                w[f"wx0{d}"] = (Wx * cs2).astype(bf16)
            else:
                # rows 0:128 multiply y0f = 2*hf, rows 128:256 multiply y0b
                w[f"wx1{d}f"] = ((Wx[0:H] * 0.5) * cs2).astype(bf16)
                w[f"wx1{d}b"] = ((Wx[H:2 * H] * 0.5) * cs2).astype(bf16)

    w["emb"] = np.asarray(inputs["emb"], f32)

    fcw = np.asarray(inputs["fc_W"], f32).copy()  # [2T, 10]
    fcw[:T] *= 0.5          # mx rows: feat carries 2*mx
    fcw[T:] *= 1.0 / 512.0  # av rows: feat carries sum(2h) over 256 feats
    w["fcw"] = fcw.astype(f32)
    w["fcb_rep"] = np.tile(np.asarray(inputs["fc_b"], f32)[None, :], (BC, 1))
    w["ident"] = np.eye(P, dtype=f32)
    w["idb"] = np.eye(P, dtype=f32).astype(bf16)
    # pooling-time partition permutation for the scan-order backward dir:
    # token partition p=(t_local*8+b) maps to p'=(15-t_local)*8+b
    pm = np.zeros((P, P), dtype=f32)
    for k in range(P):
        pm[k, (15 - k // BC) * BC + k % BC] = 1.0
    w["perm"] = pm
    return w


# ---------------------------------------------------------------------------
# device program
# ---------------------------------------------------------------------------

def _build():
    nc = bass.Bass("TRN2", target_bir_lowering=False, debug=False,
                   num_devices=NCORES)

    def di(name, shape, dtype=dt.float32):
        return nc.dram_tensor(name, shape, dtype, kind="ExternalInput")

    emb_d = di("emb", [VOCAB + 1, E])
    ident_d = di("ident", [P, P])
    idb_d = di("idb", [P, P], dt.bfloat16)
    perm_d = di("perm", [P, P])
    idx_d = di("idx", [T * BC], dt.int32)
    m2_d = di("m2", [P, T, 2, BC], dt.uint8)
    fcw_d = di("fcw", [2 * T, NCLS])
    fcb_d = di("fcb_rep", [BC, NCLS])
    wdram = {}
    for l in (0, 1):
        for d in ("f", "b"):
            wdram[f"wh{l}{d}"] = di(f"wh{l}{d}", [H, 4 * H], dt.bfloat16)
            wdram[f"bcol{l}{d}"] = di(f"bcol{l}{d}", [P, 4])
            if l == 0:
                wdram[f"wx0{d}"] = di(f"wx0{d}", [E, 4 * H], dt.bfloat16)
            else:
                wdram[f"wx1{d}f"] = di(f"wx1{d}f", [H, 4 * H], dt.bfloat16)
                wdram[f"wx1{d}b"] = di(f"wx1{d}b", [H, 4 * H], dt.bfloat16)

    out_d = nc.dram_tensor("out", [BC, NCLS], dt.float32, kind="ExternalOutput")
    feat_dram = nc.dram_tensor("feat", [2, T, BC], dt.float32)

    NTOK = T * BC            # 4096 tokens per core
    NCH = NTOK // P          # 32 gather/pool chunks
    NXC = NTOK // 512        # 8 xp matmul chunks
    TCH = 512 // BC          # 64 timesteps per xp chunk
    TPC = P // BC            # 16 timesteps per pooling chunk

    with tile.TileContext(nc) as tc:
        with (
            tc.tile_pool(name="const", bufs=1) as cpool,
            tc.tile_pool(name="work", bufs=4) as wpool,
            tc.tile_pool(name="psx", bufs=2, space="PSUM") as psx,
            tc.tile_pool(name="psz", bufs=(4 if UNROLL <= 8 else 2),
                         space="PSUM") as psz,
            tc.tile_pool(name="psf", bufs=1, space="PSUM") as psf,
        ):
            # ---- constant loads
            ident = cpool.tile([P, P], dt.float32, tag="ident")
            nc.sync.dma_start(out=ident[:], in_=ident_d[:])
            idb = cpool.tile([P, P], dt.bfloat16, tag="idb")
            nc.sync.dma_start(out=idb[:], in_=idb_d[:])
            perm = cpool.tile([P, P], dt.float32, tag="perm")
            nc.sync.dma_start(out=perm[:], in_=perm_d[:])
            idx_t = cpool.tile([P, NCH], dt.int32, tag="idx")
            nc.sync.dma_start(
                out=idx_t[:], in_=idx_d.rearrange("(c p) -> p c", p=P))
            m2 = cpool.tile([P, T, 2, BC], dt.uint8, tag="m2", name="m2")
            nc.sync.dma_start(out=m2[:], in_=m2_d[:])
            wsb = {}
            for k, dr in wdram.items():
                sh = list(dr.shape)
                wt_ = cpool.tile(sh, dr.dtype, tag=k, name=k)
                nc.sync.dma_start(out=wt_[:], in_=dr[:])
                wsb[k] = wt_
            fcw_t = cpool.tile([P, 2 * T // P, NCLS], dt.float32, tag="fcw")
            nc.sync.dma_start(
                out=fcw_t[:], in_=fcw_d.rearrange("(q p) c -> p q c", p=P))
            fcb_t = cpool.tile([BC, NCLS], dt.float32, tag="fcb")
            nc.sync.dma_start(out=fcb_t[:], in_=fcb_d[:])

            # persistent SBUF state
            xpT = cpool.tile([P, T, 4, 2, BC], dt.bfloat16, tag="xpT",
                             name="xpT")
            y0 = cpool.tile([P, 2, T, BC], dt.bfloat16, tag="y0", name="y0")
            y1 = cpool.tile([P, 2, T, BC], dt.float32, tag="y1", name="y1")
            # rolling H history for one UNROLL block; slot j = h after step j
            hwin = cpool.tile([P, UNROLL, 2, BC], dt.bfloat16, tag="hwin",
                              name="hwin")
            Cs = cpool.tile([P, 2, BC], dt.float32, tag="Cs", name="Cs")

            def xp_epilogue(l, dd, d, n, g, ps):
                """xpT[:, chunk, g, dd, :] = ps + bias_col + K_g*(1-m)."""
                t0, t1 = n * TCH, (n + 1) * TCH
                dst = xpT[:, t0:t1, g, dd, :]
                bcol = wsb[f"bcol{l}{d}"]
                kg = -KSAT if g == 0 else (KSAT if g == 1 else 0.0)
                if kg != 0.0:
                    # tmp = m*(-kg) + ps in fp32; the big +-KSAT intermediate
                    # must not round through bf16 (ulp(40) = 0.25), so only
                    # the final small-valued sum is written to the bf16 dst
                    tmp = wpool.tile([P, TCH, BC], dt.float32, tag="xptmp")
                    nc.vector.scalar_tensor_tensor(
                        out=tmp[:], in0=m2[:, t0:t1, dd, :], scalar=-kg,
                        in1=ps[:], op0=ALU.mult, op1=ALU.add)
                    nc.vector.tensor_scalar(
                        out=dst, in0=tmp[:], scalar1=bcol[:, g:g + 1],
                        scalar2=float(kg), op0=ALU.add, op1=ALU.add)
                else:
                    nc.vector.tensor_scalar(
                        out=dst, in0=ps[:], scalar1=bcol[:, g:g + 1],
                        scalar2=None, op0=ALU.add)

            # ---- embedding gather + transpose + layer-0 xp
            with tc.tile_pool(name="gph", bufs=3) as gpool, \
                 tc.tile_pool(name="gbig", bufs=1) as gbig:
                g128 = gbig.tile([P, T, BC], dt.bfloat16, tag="g128")
                g128f = g128[:].rearrange("p t b -> p (t b)")
                for c in range(NCH):
                    gr = gpool.tile([P, E], dt.float32, tag="gr")
                    nc.gpsimd.indirect_dma_start(
                        out=gr[:], out_offset=None, in_=emb_d[:],
                        in_offset=bass.IndirectOffsetOnAxis(
                            ap=idx_t[:, c:c + 1], axis=0),
                    )
                    pt = psx.tile([P, P], dt.float32, tag="psxp")
                    nc.tensor.transpose(out=pt[:], in_=gr[:], identity=ident[:])
                    nc.vector.tensor_copy(
                        out=g128f[:, c * P:(c + 1) * P], in_=pt[:])

                for dd, d, rv in ((0, "f", g128[:]), (1, "b", g128[:, ::-1, :])):
                    wxa = wsb[f"wx0{d}"]
                    for n in range(NXC):
                        t0, t1 = n * TCH, (n + 1) * TCH
                        for g in range(4):
                            ps = psx.tile([P, TCH, BC], dt.float32, tag="psxp")
                            nc.tensor.matmul(
                                out=ps[:], lhsT=wxa[:, g * H:(g + 1) * H],
                                rhs=rv[:, t0:t1, :], start=True, stop=True)
                            xp_epilogue(0, dd, d, n, g, ps)

            # ---- the merged-direction scan
            def scan_layer(l, ysb):
                nc.vector.memset(hwin[:], 0.0)
                nc.vector.memset(Cs[:], 0.0)
                whf = wsb[f"wh{l}f"]
                whb = wsb[f"wh{l}b"]
                with tc.For_i(0, T, UNROLL) as t0:
                    # one dynamic AP per engine per block (per-step ds()
                    # offsets exhaust the register file); everything inside
                    # the block indexes statically
                    xw = xpT[:, bass.ds(t0, UNROLL), :, :, :]
                    mw = m2[:, bass.ds(t0, UNROLL), :, :]
                    yw = ysb[:, :, bass.ds(t0, UNROLL), :]
                    # stage the block's xp in SBUF (only the DVE handles
                    # dynamic APs); each step injects it into PSUM via an
                    # identity matmul so the accumulation group is pure-PE
                    # (mixed DVE-write + PE-accumulate on a PSUM bank races)
                    xstage = wpool.tile([P, UNROLL, 4, 2, BC], dt.bfloat16,
                                        tag="xstage")
                    nc.vector.tensor_copy(out=xstage[:], in_=xw)
                    zsblk = psz.tile([P, UNROLL, 4, 2, BC], dt.float32,
                                     tag="zs")
                    mwin = wpool.tile([P, UNROLL, 2, BC], dt.uint8,
                                      tag="mwin")
                    nc.vector.tensor_copy(out=mwin[:], in_=mw)
                    # block-wide identity matmuls inject xp for all UNROLL
                    # steps (start=True; <=512 free elements per matmul)
                    zsf = zsblk[:].rearrange("p u g d b -> p (u g d b)")
                    xsf = xstage[:].rearrange("p u g d b -> p (u g d b)")
                    SPW = 512 // (4 * 2 * BC)  # steps per psum-bank matmul
                    for s0 in range(0, UNROLL, SPW):
                        lo, hi = s0 * 64, (s0 + SPW) * 64
                        nc.tensor.matmul(
                            out=zsf[:, lo:hi], lhsT=idb[:],
                            rhs=xsf[:, lo:hi],
                            start=True, stop=False, skip_group_check=True)
                    for j in range(UNROLL):
                        jp = j - 1 if j > 0 else UNROLL - 1
                        for dd, wh in ((0, whf), (1, whb)):
                            for g in range(4):
                                nc.tensor.matmul(
                                    out=zsblk[:, j, g, dd, :],
                                    lhsT=wh[:, g * H:(g + 1) * H],
                                    rhs=hwin[:, jp, dd, :],
                                    start=False, stop=(dd == 1 and g == 3),
                                    skip_group_check=True)
                        tall = wpool.tile([P, 4, 2, BC], dt.float32, tag="tall")
                        nc.scalar.activation(
                            out=tall[:], in_=zsblk[:, j, :, :, :],
                            func=AF.Tanh, scale=0.5)
                        wt = wpool.tile([P, 2, BC], dt.float32, tag="wt")
                        nc.vector.scalar_tensor_tensor(
                            out=wt[:], in0=tall[:, 0, :, :], scalar=1.0,
                            in1=tall[:, 2, :, :], op0=ALU.add, op1=ALU.mult)
                        pt_ = wpool.tile([P, 2, BC], dt.float32, tag="pt")
                        nc.vector.scalar_tensor_tensor(
                            out=pt_[:], in0=tall[:, 1, :, :], scalar=1.0,
                            in1=Cs[:], op0=ALU.add, op1=ALU.mult)
                        nc.vector.scalar_tensor_tensor(
                            out=Cs[:], in0=pt_[:], scalar=0.5,
                            in1=wt[:], op0=ALU.mult, op1=ALU.add)
                        tct = wpool.tile([P, 2, BC], dt.float32, tag="tct")
                        nc.scalar.activation(
                            out=tct[:], in_=Cs[:], func=AF.Tanh, scale=0.5)
                        rt = wpool.tile([P, 2, BC], dt.bfloat16, tag="rt")
                        nc.vector.scalar_tensor_tensor(
                            out=rt[:], in0=tall[:, 3, :, :], scalar=1.0,
                            in1=tct[:], op0=ALU.add, op1=ALU.mult)
                        # masked lanes carry h_{t-1}: seed slot j with the
                        # previous h, then overwrite unmasked lanes
                        nc.scalar.copy(out=hwin[:, j, :, :],
                                       in_=hwin[:, jp, :, :])
                        nc.vector.copy_predicated(
                            out=hwin[:, j, :, :],
                            mask=mwin[:, j, :, :], data=rt[:])
                    nc.scalar.copy(
                        out=yw, in_=hwin[:].rearrange("p u d b -> p d u b"))

            scan_layer(0, y0)

            # ---- layer-1 xp from SBUF y0 (reversed views for time flips)
            # dir f at fwd t: y0f straight, y0b slice reversed
            # dir b at scan s: y0f slice reversed, y0b straight
            y0r = y0[:, :, ::-1, :]
            for dd, d in ((0, "f"), (1, "b")):
                for n in range(NXC):
                    t0, t1 = n * TCH, (n + 1) * TCH
                    if dd == 0:
                        rf = y0[:, 0, t0:t1, :]
                        rb = y0r[:, 1, t0:t1, :]
                    else:
                        rf = y0r[:, 0, t0:t1, :]
                        rb = y0[:, 1, t0:t1, :]
                    for g in range(4):
                        ps = psx.tile([P, TCH, BC], dt.float32, tag="psxp")
                        nc.tensor.matmul(
                            out=ps[:],
                            lhsT=wsb[f"wx1{d}f"][:, g * H:(g + 1) * H],
                            rhs=rf, start=True, stop=False)
                        nc.tensor.matmul(
                            out=ps[:],
                            lhsT=wsb[f"wx1{d}b"][:, g * H:(g + 1) * H],
                            rhs=rb, start=False, stop=True)
                        xp_epilogue(1, dd, d, n, g, ps)

            scan_layer(1, y1)

            # ---- pooling: PE-transpose [h -> token] chunks, reduce over 2H
            with tc.tile_pool(name="ep", bufs=4) as epool:
                fmxf = cpool.tile([P, NCH], dt.float32, tag="fmxf")
                fsmf = cpool.tile([P, NCH], dt.float32, tag="fsmf")
                fmxb = cpool.tile([P, NCH], dt.float32, tag="fmxb")
                fsmb = cpool.tile([P, NCH], dt.float32, tag="fsmb")
                for c in range(NCH):
                    tp = psx.tile([P, 2, P], dt.float32, tag="psxp")
                    for dd in (0, 1):
                        nc.tensor.transpose(
                            out=tp[:, dd, :],
                            in_=y1[:, dd, c * TPC:(c + 1) * TPC, :],
                            identity=ident[:])
                    # backward dir is scan-order: chunk c is fwd chunk 31-c
                    cb = NCH - 1 - c
                    nc.vector.tensor_reduce(
                        out=fmxf[:, c:c + 1], in_=tp[:, 0, :],
                        axis=mybir.AxisListType.XYZW, op=ALU.max)
                    nc.vector.tensor_reduce(
                        out=fsmf[:, c:c + 1], in_=tp[:, 0, :],
                        axis=mybir.AxisListType.XYZW, op=ALU.add)
                    nc.vector.tensor_reduce(
                        out=fmxb[:, cb:cb + 1], in_=tp[:, 1, :],
                        axis=mybir.AxisListType.XYZW, op=ALU.max)
                    nc.vector.tensor_reduce(
                        out=fsmb[:, cb:cb + 1], in_=tp[:, 1, :],
                        axis=mybir.AxisListType.XYZW, op=ALU.add)
                # permute backward-dir partitions to forward token order
                pmx = psf.tile([P, 2, NCH], dt.float32, tag="pfeat")
                nc.tensor.matmul(out=pmx[:, 0, :], lhsT=perm[:], rhs=fmxb[:],
                                 start=True, stop=True)
                nc.tensor.matmul(out=pmx[:, 1, :], lhsT=perm[:], rhs=fsmb[:],
                                 start=True, stop=True)
                fmx = epool.tile([P, NCH], dt.float32, tag="fmx")
                nc.vector.tensor_tensor(
                    out=fmx[:], in0=fmxf[:], in1=pmx[:, 0, :], op=ALU.max)
                fsum = epool.tile([P, NCH], dt.float32, tag="fsum")
                nc.vector.tensor_tensor(
                    out=fsum[:], in0=fsmf[:], in1=pmx[:, 1, :], op=ALU.add)
                featv = feat_dram.rearrange("s t b -> s (t b)")
                nc.sync.dma_start(
                    out=featv[0].rearrange("(c p) -> p c", p=P), in_=fmx[:])
                nc.sync.dma_start(
                    out=featv[1].rearrange("(c p) -> p c", p=P), in_=fsum[:])

                # ---- FC head: out = relu(featT.T @ fcw + b)
                pfc = psf.tile([BC, NCLS], dt.float32, tag="pfc")
                NQ = 2 * T // P
                for q in range(NQ):
                    lq = epool.tile([P, BC], dt.float32, tag="lq")
                    pool_i, tq = divmod(q * P, T)
                    nc.sync.dma_start(
                        out=lq[:], in_=feat_dram[pool_i, tq:tq + P, :])
                    nc.tensor.matmul(
                        out=pfc[:], lhsT=lq[:], rhs=fcw_t[:, q, :],
                        start=(q == 0), stop=(q == NQ - 1))
                ob = epool.tile([BC, NCLS], dt.float32, tag="ob")
                nc.vector.tensor_tensor(
                    out=ob[:], in0=pfc[:], in1=fcb_t[:], op=ALU.add)
                nc.vector.tensor_scalar(
                    out=ob[:], in0=ob[:], scalar1=0.0, scalar2=None,
                    op0=ALU.max)
                nc.sync.dma_start(out=out_d[:], in_=ob[:])

    split_multi_waits(nc)
    return nc


_cached_nc = None


def _get_nc():
    global _cached_nc
    if _cached_nc is None:
        _install_hook()
        _cached_nc = _build()
    return _cached_nc


def _in_maps(inputs):
    w = _fold_weights(inputs)
    x = np.asarray(inputs["x"]).astype(np.int32)  # [64, 512]
    shared = {
        "emb": w["emb"], "ident": w["ident"], "idb": w["idb"],
        "perm": w["perm"], "fcw": w["fcw"], "fcb_rep": w["fcb_rep"],
    }
    for l in (0, 1):
        for d in ("f", "b"):
            shared[f"wh{l}{d}"] = w[f"wh{l}{d}"]
            shared[f"bcol{l}{d}"] = w[f"bcol{l}{d}"]
            if l == 0:
                shared[f"wx0{d}"] = w[f"wx0{d}"]
            else:
                shared[f"wx1{d}f"] = w[f"wx1{d}f"]
                shared[f"wx1{d}b"] = w[f"wx1{d}b"]
    maps = []
    for c in range(NCORES):
        xc = x[c * BC:(c + 1) * BC]            # [BC, T]
        idx = np.ascontiguousarray(xc.T).reshape(-1).astype(np.int32)
        m = (xc != 0).astype(np.uint8).T       # [T, BC]
        m2 = np.stack([m, m[::-1]], axis=1)    # [T, 2, BC]
        m2 = np.broadcast_to(m2[None], (P, T, 2, BC))
        maps.append(dict(shared, idx=idx, m2=np.ascontiguousarray(m2)))
    return maps


def _run(inputs, trace=False):
    from concourse.bass_utils import run_bass_kernel_spmd
    nc = _get_nc()
    maps = _in_maps(inputs)
    res = run_bass_kernel_spmd(nc, maps, list(range(NCORES)), trace=trace)
    out = np.concatenate([res.results[c]["out"] for c in range(NCORES)], axis=0)
    return out.astype(np.float32), res


def kernel(**inputs):
    out, _ = _run(inputs, trace=False)
    return out


def run_traced(inputs):
    out, res = _run(inputs, trace=True)
    return out, res
